# revision 4
# baseline (speedup 1.0000x reference)
"""Trainium2 Bass kernel for the two-branch sparse-attention fusion module.

Math (per batch b, tokens T = rgb/evt reshaped to (d=256, N=4096)):
    s      = sum_n T[:, n]                     (256,)
    k_sum  = Wk @ s + N*bk                     (256,)
    v      = Wq^T @ k_sum = (Wk^T Wq)^T... -> value[n] = T[:,n].v + c
    value  = (T^T v + c) / sqrt(d)
    w      = sigmoid(value_rgb - value_evt)    (softmax over 2 == sigmoid of diff)
    out    = evt + w * (rgb - evt)

Sharded data-parallel over batch: 8 cores x 2 batches, weights replicated.
"""

import numpy as np
from contextlib import ExitStack

import concourse.bass as bass
import concourse.tile as tile
from concourse import bacc, mybir
from concourse.bass_utils import run_bass_kernel_spmd

F32 = mybir.dt.float32

BS, DIM, HH, WW = 16, 256, 64, 64
N = HH * WW                 # 4096 tokens
NCORES = 8
BPC = BS // NCORES          # batches per core
PH = DIM // 128             # partition halves of the d dim
CH = 512                    # n-chunk (one PSUM bank of f32)
NCH = N // CH
INV_SQRT_D = 1.0 / 16.0


def build_nc() -> bass.Bass:
    nc = bacc.Bacc()

    rgb = nc.declare_dram_parameter("rgb", [BPC, PH, 128, N], F32, isOutput=False)
    evt = nc.declare_dram_parameter("evt", [BPC, PH, 128, N], F32, isOutput=False)
    wts = {}
    for nm in ("Wq_a", "Wk_a", "Wq_d", "Wk_d"):
        wts[nm] = nc.declare_dram_parameter(nm, [PH, 128, DIM], F32, isOutput=False)
    bss = {}
    for nm in ("bq_a", "bk_a", "bq_d", "bk_d"):
        bss[nm] = nc.declare_dram_parameter(nm, [PH, 128, 1], F32, isOutput=False)
    out = nc.declare_dram_parameter("out", [BPC, PH, 128, N], F32, isOutput=True)

    with tile.TileContext(nc) as tc:
        _body(tc, rgb, evt, wts, bss, out)
    nc.finalize()
    return nc


def _body(tc, rgb, evt, wts, bss, out):
    nc = tc.nc
    with ExitStack() as ctx:
        consts = ctx.enter_context(tc.tile_pool(name="consts", bufs=1))
        data = ctx.enter_context(tc.tile_pool(name="data", bufs=2))
        small = ctx.enter_context(tc.tile_pool(name="small", bufs=2))
        wrow_p = ctx.enter_context(tc.tile_pool(name="wrow", bufs=2))
        ps_val = ctx.enter_context(tc.tile_pool(name="ps_val", bufs=2, space="PSUM"))
        ps_wb = ctx.enter_context(tc.tile_pool(name="ps_wb", bufs=2, space="PSUM"))
        ps_sm = ctx.enter_context(tc.tile_pool(name="ps_sm", bufs=2, space="PSUM"))

        # ---- load weights + biases -------------------------------------
        W = {}  # (name, o_half) -> (128, 256) sbuf
        for nm in ("Wq_a", "Wk_a", "Wq_d", "Wk_d"):
            for h in range(PH):
                t = consts.tile([128, DIM], F32, tag=f"{nm}{h}")
                nc.sync.dma_start(out=t, in_=wts[nm][h])
                W[(nm, h)] = t
        B = {}  # (name, o_half) -> (128, 1) sbuf
        for nm in ("bq_a", "bk_a", "bq_d", "bk_d"):
            for h in range(PH):
                t = consts.tile([128, 1], F32, tag=f"{nm}{h}")
                nc.sync.dma_start(out=t, in_=bss[nm][h])
                B[(nm, h)] = t

        ones_row = consts.tile([1, 128], F32, tag="ones")
        nc.vector.memset(ones_row, 1.0)

        # ---- weight precompute (sign folds the a-minus-d diff) ---------
        # PT[br][jh] = sign * (Wk^T Wq)[j, :]   (lhsT for the v matvec)
        # U[br][ih]  = sign * N * (Wq^T bk)[i]
        # R[br][jh]  = sign * (Wk^T bq)[j]
        # BKN[br][oh]= sign * N * bk[o]
        PT, U, R, BKN = {}, {}, {}, {}
        for br, wq, wk, sign in (
            ("a", "Wq_a", "Wk_a", 1.0),
            ("d", "Wq_d", "Wk_d", -1.0),
        ):
            for jh in range(PH):
                ps = ps_sm.tile([128, DIM], F32, tag="ps_sm")
                for oh in range(PH):
                    nc.tensor.matmul(
                        ps,
                        lhsT=W[(wk, oh)][:, jh * 128 : (jh + 1) * 128],
                        rhs=W[(wq, oh)],
                        start=(oh == 0),
                        stop=(oh == PH - 1),
                    )
                t = consts.tile([128, DIM], F32, tag=f"PT{br}{jh}")
                nc.scalar.mul(out=t, in_=ps, mul=sign)
                PT[(br, jh)] = t
            for ih in range(PH):
                ps = ps_sm.tile([128, 1], F32, tag="ps_sm")
                for oh in range(PH):
                    nc.tensor.matmul(
                        ps,
                        lhsT=W[(wq, oh)][:, ih * 128 : (ih + 1) * 128],
                        rhs=B[("bk_" + br, oh)],
                        start=(oh == 0),
                        stop=(oh == PH - 1),
                    )
                t = consts.tile([128, 1], F32, tag=f"U{br}{ih}")
                nc.scalar.mul(out=t, in_=ps, mul=sign * N)
                U[(br, ih)] = t
            for jh in range(PH):
                ps = ps_sm.tile([128, 1], F32, tag="ps_sm")
                for oh in range(PH):
                    nc.tensor.matmul(
                        ps,
                        lhsT=W[(wk, oh)][:, jh * 128 : (jh + 1) * 128],
                        rhs=B[("bq_" + br, oh)],
                        start=(oh == 0),
                        stop=(oh == PH - 1),
                    )
                t = consts.tile([128, 1], F32, tag=f"R{br}{jh}")
                nc.scalar.mul(out=t, in_=ps, mul=sign)
                R[(br, jh)] = t
            for oh in range(PH):
                t = consts.tile([128, 1], F32, tag=f"BKN{br}{oh}")
                nc.scalar.mul(out=t, in_=B[("bk_" + br, oh)], mul=sign * N)
                BKN[(br, oh)] = t

        # ---- per-batch pipeline ----------------------------------------
        for b in range(BPC):
            A, Dv = {}, {}
            for h in range(PH):
                tA = data.tile([128, N], F32, tag=f"A{h}")
                nc.sync.dma_start(out=tA, in_=rgb[b, h])
                A[h] = tA
                tD = data.tile([128, N], F32, tag=f"D{h}")
                nc.sync.dma_start(out=tD, in_=evt[b, h])
                Dv[h] = tD

            # s = row sums of the tokens
            S = {}
            for h in range(PH):
                sa = small.tile([128, 1], F32, tag=f"sa{h}")
                nc.vector.reduce_sum(out=sa, in_=A[h], axis=mybir.AxisListType.X)
                S[("a", h)] = sa
                sd = small.tile([128, 1], F32, tag=f"sd{h}")
                nc.vector.reduce_sum(out=sd, in_=Dv[h], axis=mybir.AxisListType.X)
                S[("d", h)] = sd

            # v = PT @ s + U  per branch (d branch carries the minus sign)
            V = {}
            for br in ("a", "d"):
                for ih in range(PH):
                    ps = ps_sm.tile([128, 1], F32, tag="ps_sm")
                    for jh in range(PH):
                        nc.tensor.matmul(
                            ps,
                            lhsT=PT[(br, jh)][:, ih * 128 : (ih + 1) * 128],
                            rhs=S[(br, jh)],
                            start=(jh == 0),
                            stop=(jh == PH - 1),
                        )
                    v = small.tile([128, 1], F32, tag=f"v{br}{ih}")
                    nc.vector.tensor_add(out=v, in0=ps, in1=U[(br, ih)])
                    V[(br, ih)] = v

            # c_diff = sum of tiny dot products (both branches + bias terms)
            ps_c = ps_sm.tile([1, 1], F32, tag="ps_sm")
            terms = (
                [(S[("a", jh)], R[("a", jh)]) for jh in range(PH)]
                + [(S[("d", jh)], R[("d", jh)]) for jh in range(PH)]
                + [(B[("bq_a", oh)], BKN[("a", oh)]) for oh in range(PH)]
                + [(B[("bq_d", oh)], BKN[("d", oh)]) for oh in range(PH)]
            )
            for i, (l, r) in enumerate(terms):
                nc.tensor.matmul(
                    ps_c, lhsT=l, rhs=r, start=(i == 0), stop=(i == len(terms) - 1)
                )
            c16 = small.tile([1, 1], F32, tag="c16")
            nc.scalar.mul(out=c16, in_=ps_c, mul=INV_SQRT_D)

            # per n-chunk: value diff -> sigmoid -> broadcast -> blend
            wrow = wrow_p.tile([1, N], F32, tag="wrow")
            for ich in range(NCH):
                sl = slice(ich * CH, (ich + 1) * CH)
                psv = ps_val.tile([1, CH], F32, tag="psv")
                mms = [
                    (V[("a", 0)], A[0]),
                    (V[("a", 1)], A[1]),
                    (V[("d", 0)], Dv[0]),
                    (V[("d", 1)], Dv[1]),
                ]
                for i, (v, t) in enumerate(mms):
                    nc.tensor.matmul(
                        psv, lhsT=v, rhs=t[:, sl],
                        start=(i == 0), stop=(i == len(mms) - 1),
                    )
                nc.scalar.activation(
                    out=wrow[:, sl], in_=psv,
                    func=mybir.ActivationFunctionType.Sigmoid,
                    bias=c16, scale=INV_SQRT_D,
                )
                wb = ps_wb.tile([128, CH], F32, tag="wb")
                nc.tensor.matmul(wb, lhsT=ones_row, rhs=wrow[:, sl], start=True, stop=True)
                for h in range(PH):
                    nc.vector.tensor_sub(out=A[h][:, sl], in0=A[h][:, sl], in1=Dv[h][:, sl])
                    nc.vector.tensor_mul(out=A[h][:, sl], in0=A[h][:, sl], in1=wb)
                    nc.vector.tensor_add(out=A[h][:, sl], in0=A[h][:, sl], in1=Dv[h][:, sl])

            for h in range(PH):
                nc.sync.dma_start(out=out[b, h], in_=A[h])


_NC_CACHE = None


def _get_nc():
    global _NC_CACHE
    if _NC_CACHE is None:
        _NC_CACHE = build_nc()
    return _NC_CACHE


def _make_in_maps(inputs):
    rgb = np.ascontiguousarray(np.asarray(inputs["rgb"], dtype=np.float32)).reshape(
        BS, PH, 128, N
    )
    evt = np.ascontiguousarray(np.asarray(inputs["evt"], dtype=np.float32)).reshape(
        BS, PH, 128, N
    )
    base = {}
    for nm in ("Wq_a", "Wk_a", "Wq_d", "Wk_d"):
        base[nm] = np.ascontiguousarray(
            np.asarray(inputs[nm], dtype=np.float32)
        ).reshape(PH, 128, DIM)
    for nm in ("bq_a", "bk_a", "bq_d", "bk_d"):
        base[nm] = np.ascontiguousarray(
            np.asarray(inputs[nm], dtype=np.float32)
        ).reshape(PH, 128, 1)
    in_maps = []
    for c in range(NCORES):
        m = dict(base)
        m["rgb"] = np.ascontiguousarray(rgb[c * BPC : (c + 1) * BPC])
        m["evt"] = np.ascontiguousarray(evt[c * BPC : (c + 1) * BPC])
        in_maps.append(m)
    return in_maps


def run(inputs, trace=False):
    nc = _get_nc()
    in_maps = _make_in_maps(inputs)
    res = run_bass_kernel_spmd(nc, in_maps, core_ids=list(range(NCORES)), trace=trace)
    outs = [
        np.asarray(res.results[i]["out"]).reshape(BPC, DIM, HH, WW)
        for i in range(NCORES)
    ]
    full = np.concatenate(outs, axis=0)
    return full, res


def kernel(**inputs) -> np.ndarray:
    full, _ = run(inputs, trace=False)
    return full


# revision 6
# speedup vs baseline: 1.2385x; 1.2385x over previous
"""Trainium2 Bass kernel for the two-branch sparse-attention fusion module.

Math (per batch b, tokens T = rgb/evt as (d=256, N=4096) d-major):
    s      = sum_n T[:, n]                           (256,)
    value[n] = T[:,n].v + c, v = (Wk^T Wq)^T s + N Wq^T bk, c = (Wk^T bq).s + N bq.bk
    w      = sigmoid((value_rgb - value_evt)/sqrt(d))
    out    = evt + w * (rgb - evt)

Engine split per batch:
    DMA   : 4x (128,1024) block loads per tile, 2048-col block stores
    ScalarE: streaming row-sum partials (Copy+accum_out), sigmoid (bf16),
             wb PSUM->SBUF bf16 copies
    PE    : tiny weight-product matvecs, fp32 value matmuls (4-chunk PSUM
            accumulation), bf16 K=1 broadcast of w to 128 partitions
    DVE   : M=A-D (f32->bf16), M*=wb (bf16 2x mode), A=M+D (f32 out)

Sharded data-parallel over batch: 8 cores x 2 batches, weights replicated.
"""

import numpy as np
from contextlib import ExitStack

import concourse.bass as bass
import concourse.tile as tile
from concourse import bacc, mybir
from concourse.bass_utils import run_bass_kernel_spmd

F32 = mybir.dt.float32
BF16 = mybir.dt.bfloat16

BS, DIM, HH, WW = 16, 256, 64, 64
N = HH * WW                 # 4096 tokens
NCORES = 8
BPC = BS // NCORES          # batches per core
PH = DIM // 128             # partition halves of the d dim
CH = 512                    # value-chunk (one PSUM bank of f32)
NCH = N // CH               # 8
LB = 1024                   # load block columns
NLB = N // LB               # 4
SB = 2048                   # store/blend block columns
NSB = N // SB               # 2
INV_SQRT_D = 1.0 / 16.0


def build_nc() -> bass.Bass:
    nc = bacc.Bacc()

    rgb = nc.declare_dram_parameter("rgb", [BPC, PH, 128, N], F32, isOutput=False)
    evt = nc.declare_dram_parameter("evt", [BPC, PH, 128, N], F32, isOutput=False)
    wts = {}
    for nm in ("Wq_a", "Wk_a", "Wq_d", "Wk_d"):
        wts[nm] = nc.declare_dram_parameter(nm, [PH, 128, DIM], F32, isOutput=False)
    bss = {}
    for nm in ("bq_a", "bk_a", "bq_d", "bk_d"):
        bss[nm] = nc.declare_dram_parameter(nm, [PH, 128, 1], F32, isOutput=False)
    out = nc.declare_dram_parameter("out", [BPC, PH, 128, N], F32, isOutput=True)

    with tile.TileContext(nc) as tc:
        _body(tc, rgb, evt, wts, bss, out)
    nc.finalize()
    return nc


def _body(tc, rgb, evt, wts, bss, out):
    nc = tc.nc
    ACT = mybir.ActivationFunctionType
    with ExitStack() as ctx:
        consts = ctx.enter_context(tc.tile_pool(name="consts", bufs=1))
        data = ctx.enter_context(tc.tile_pool(name="data", bufs=2))
        mpool = ctx.enter_context(tc.tile_pool(name="mpool", bufs=1))
        wbp = ctx.enter_context(tc.tile_pool(name="wbp", bufs=2))
        small = ctx.enter_context(tc.tile_pool(name="small", bufs=2))
        wchunk = ctx.enter_context(tc.tile_pool(name="wchunk", bufs=4))
        ps_val = ctx.enter_context(tc.tile_pool(name="ps_val", bufs=3, space="PSUM"))
        ps_wb = ctx.enter_context(tc.tile_pool(name="ps_wb", bufs=2, space="PSUM"))
        ps_sm = ctx.enter_context(tc.tile_pool(name="ps_sm", bufs=2, space="PSUM"))

        # ---- load weights + biases -------------------------------------
        W = {}  # (name, o_half) -> (128, 256) sbuf
        for nm in ("Wq_a", "Wk_a", "Wq_d", "Wk_d"):
            for h in range(PH):
                t = consts.tile([128, DIM], F32, tag=f"{nm}{h}")
                nc.sync.dma_start(out=t, in_=wts[nm][h])
                W[(nm, h)] = t
        B = {}  # (name, o_half) -> (128, 1) sbuf
        for nm in ("bq_a", "bk_a", "bq_d", "bk_d"):
            for h in range(PH):
                t = consts.tile([128, 1], F32, tag=f"{nm}{h}")
                nc.sync.dma_start(out=t, in_=bss[nm][h])
                B[(nm, h)] = t

        ones_row = consts.tile([1, 128], BF16, tag="ones")
        nc.vector.memset(ones_row, 1.0)
        one_one = consts.tile([1, 1], F32, tag="one_one")
        nc.vector.memset(one_one, 1.0)
        garbage = consts.tile([128, 1], F32, tag="garbage")

        # ---- weight precompute (sign folds the a-minus-d diff) ---------
        # PT[br][jh] = sign * (Wk^T Wq)[j, :]   (lhsT for the v matvec)
        # U[br][ih]  = sign * N * (Wq^T bk)[i]
        # R[br][jh]  = sign * (Wk^T bq)[j]
        # c_bias     = N * (bq_a.bk_a - bq_d.bk_d)
        PT, U, R = {}, {}, {}
        for br, wq, wk, sign in (
            ("a", "Wq_a", "Wk_a", 1.0),
            ("d", "Wq_d", "Wk_d", -1.0),
        ):
            for jh in range(PH):
                ps = ps_sm.tile([128, DIM], F32, tag="ps_sm")
                for oh in range(PH):
                    nc.tensor.matmul(
                        ps,
                        lhsT=W[(wk, oh)][:, jh * 128 : (jh + 1) * 128],
                        rhs=W[(wq, oh)],
                        start=(oh == 0),
                        stop=(oh == PH - 1),
                    )
                t = consts.tile([128, DIM], F32, tag=f"PT{br}{jh}")
                nc.scalar.mul(out=t, in_=ps, mul=sign)
                PT[(br, jh)] = t
            for ih in range(PH):
                ps = ps_sm.tile([128, 1], F32, tag="ps_sm")
                for oh in range(PH):
                    nc.tensor.matmul(
                        ps,
                        lhsT=W[(wq, oh)][:, ih * 128 : (ih + 1) * 128],
                        rhs=B[("bk_" + br, oh)],
                        start=(oh == 0),
                        stop=(oh == PH - 1),
                    )
                t = consts.tile([128, 1], F32, tag=f"U{br}{ih}")
                nc.scalar.mul(out=t, in_=ps, mul=sign * N)
                U[(br, ih)] = t
            for jh in range(PH):
                ps = ps_sm.tile([128, 1], F32, tag="ps_sm")
                for oh in range(PH):
                    nc.tensor.matmul(
                        ps,
                        lhsT=W[(wk, oh)][:, jh * 128 : (jh + 1) * 128],
                        rhs=B[("bq_" + br, oh)],
                        start=(oh == 0),
                        stop=(oh == PH - 1),
                    )
                t = consts.tile([128, 1], F32, tag=f"R{br}{jh}")
                nc.scalar.mul(out=t, in_=ps, mul=sign)
                R[(br, jh)] = t

        # batch-independent bias-dot part of c_diff: N*(bq_a.bk_a - bq_d.bk_d)
        ps = ps_sm.tile([1, 1], F32, tag="ps_sm")
        cterms = [("bq_a", "bk_a", 1), ("bq_d", "bk_d", -1)]
        k = 0
        for bq, bk, sgn in cterms:
            for oh in range(PH):
                # fold sign*N via a prescaled copy of bk
                t = consts.tile([128, 1], F32, tag=f"bkN{bk}{oh}{sgn}")
                nc.scalar.mul(out=t, in_=B[(bk, oh)], mul=float(sgn * N))
                nc.tensor.matmul(
                    ps, lhsT=B[(bq, oh)], rhs=t, start=(k == 0), stop=(k == 3)
                )
                k += 1
        c_bias = consts.tile([1, 1], F32, tag="c_bias")
        nc.scalar.copy(out=c_bias, in_=ps)

        # ---- per-batch pipeline ----------------------------------------
        for b in range(BPC):
            A, Dv, S4 = {}, {}, {}
            for h in range(PH):
                A[h] = data.tile([128, N], F32, tag=f"A{h}", name=f"A{h}_{b}")
                Dv[h] = data.tile([128, N], F32, tag=f"D{h}", name=f"D{h}_{b}")
                for blk in range(NLB):
                    sl = slice(blk * LB, (blk + 1) * LB)
                    nc.sync.dma_start(out=A[h][:, sl], in_=rgb[b, h][:, sl])
                    nc.sync.dma_start(out=Dv[h][:, sl], in_=evt[b, h][:, sl])

            # streaming row-sum partials on ScalarE: s4[:, blk] = sum(blk)
            for key, tiles in (("a", A), ("d", Dv)):
                for h in range(PH):
                    s4 = small.tile([128, NLB], F32, tag=f"s4{key}{h}", name=f"s4{key}{h}_{b}")
                    S4[(key, h)] = s4
                    for blk in range(NLB):
                        sl = slice(blk * LB, (blk + 1) * LB)
                        nc.scalar.activation(
                            out=garbage.broadcast_to([128, LB]),
                            in_=tiles[h][:, sl],
                            func=ACT.Copy,
                            accum_out=s4[:, blk : blk + 1],
                        )
            # combine partials: s = sum over the 4 partials
            S = {}
            for key in ("a", "d"):
                for h in range(PH):
                    s = small.tile([128, 1], F32, tag=f"s{key}{h}")
                    nc.vector.reduce_sum(
                        out=s, in_=S4[(key, h)], axis=mybir.AxisListType.X
                    )
                    S[(key, h)] = s

            # v = PT @ s + U  per branch (d branch carries the minus sign)
            V = {}
            for br in ("a", "d"):
                for ih in range(PH):
                    ps = ps_sm.tile([128, 1], F32, tag="ps_sm")
                    for jh in range(PH):
                        nc.tensor.matmul(
                            ps,
                            lhsT=PT[(br, jh)][:, ih * 128 : (ih + 1) * 128],
                            rhs=S[(br, jh)],
                            start=(jh == 0),
                            stop=(jh == PH - 1),
                        )
                    v = small.tile([128, 1], F32, tag=f"v{br}{ih}")
                    nc.vector.tensor_add(out=v, in0=ps, in1=U[(br, ih)])
                    V[(br, ih)] = v

            # c_diff = sum_j r[j] s[j] (both branches) + c_bias
            ps_c = ps_sm.tile([1, 1], F32, tag="ps_sm")
            terms = [(S[(br, jh)], R[(br, jh)]) for br in ("a", "d") for jh in range(PH)]
            for i, (l, r) in enumerate(terms):
                nc.tensor.matmul(ps_c, lhsT=l, rhs=r, start=(i == 0), stop=False)
            nc.tensor.matmul(ps_c, lhsT=c_bias, rhs=one_one, start=False, stop=True)
            c16 = small.tile([1, 1], F32, tag="c16")
            nc.scalar.mul(out=c16, in_=ps_c, mul=INV_SQRT_D)

            # value diff per 512-chunk: 4 accumulated fp32 matmuls
            val_ps = []
            for ich in range(NCH):
                sl = slice(ich * CH, (ich + 1) * CH)
                psv = ps_val.tile([1, CH], F32, tag="psv")
                mms = [
                    (V[("a", 0)], A[0]),
                    (V[("a", 1)], A[1]),
                    (V[("d", 0)], Dv[0]),
                    (V[("d", 1)], Dv[1]),
                ]
                for i, (v, t) in enumerate(mms):
                    nc.tensor.matmul(
                        psv, lhsT=v, rhs=t[:, sl],
                        start=(i == 0), stop=(i == len(mms) - 1),
                    )
                val_ps.append(psv)

            # sigmoid -> bf16 w rows; broadcast via K=1 bf16 matmul; stage to SBUF
            wb_sb = wbp.tile([128, N], BF16, tag="wb_sb")
            for ich in range(NCH):
                wrow = wchunk.tile([1, CH], BF16, tag="wrow")
                nc.scalar.activation(
                    out=wrow, in_=val_ps[ich],
                    func=ACT.Sigmoid, bias=c16, scale=INV_SQRT_D,
                )
                wb = ps_wb.tile([128, CH], F32, tag="wb")
                nc.tensor.matmul(wb, lhsT=ones_row, rhs=wrow, start=True, stop=True)
                nc.scalar.copy(
                    out=wb_sb[:, ich * CH : (ich + 1) * CH], in_=wb
                )

            # blend: M = A - D (bf16), M *= wb (bf16 2x), A = M + D (f32), store
            for h in range(PH):
                M = mpool.tile([128, N], BF16, tag=f"M{h}")
                for sb in range(NSB):
                    sl = slice(sb * SB, (sb + 1) * SB)
                    nc.vector.tensor_sub(out=M[:, sl], in0=A[h][:, sl], in1=Dv[h][:, sl])
                for sb in range(NSB):
                    sl = slice(sb * SB, (sb + 1) * SB)
                    nc.vector.tensor_mul(out=M[:, sl], in0=M[:, sl], in1=wb_sb[:, sl])
                for sb in range(NSB):
                    sl = slice(sb * SB, (sb + 1) * SB)
                    nc.vector.tensor_add(out=A[h][:, sl], in0=M[:, sl], in1=Dv[h][:, sl])
                    nc.sync.dma_start(out=out[b, h][:, sl], in_=A[h][:, sl])


_NC_CACHE = None


def _get_nc():
    global _NC_CACHE
    if _NC_CACHE is None:
        _NC_CACHE = build_nc()
    return _NC_CACHE


def _make_in_maps(inputs):
    rgb = np.ascontiguousarray(np.asarray(inputs["rgb"], dtype=np.float32)).reshape(
        BS, PH, 128, N
    )
    evt = np.ascontiguousarray(np.asarray(inputs["evt"], dtype=np.float32)).reshape(
        BS, PH, 128, N
    )
    base = {}
    for nm in ("Wq_a", "Wk_a", "Wq_d", "Wk_d"):
        base[nm] = np.ascontiguousarray(
            np.asarray(inputs[nm], dtype=np.float32)
        ).reshape(PH, 128, DIM)
    for nm in ("bq_a", "bk_a", "bq_d", "bk_d"):
        base[nm] = np.ascontiguousarray(
            np.asarray(inputs[nm], dtype=np.float32)
        ).reshape(PH, 128, 1)
    in_maps = []
    for c in range(NCORES):
        m = dict(base)
        m["rgb"] = np.ascontiguousarray(rgb[c * BPC : (c + 1) * BPC])
        m["evt"] = np.ascontiguousarray(evt[c * BPC : (c + 1) * BPC])
        in_maps.append(m)
    return in_maps


def run(inputs, trace=False):
    nc = _get_nc()
    in_maps = _make_in_maps(inputs)
    res = run_bass_kernel_spmd(nc, in_maps, core_ids=list(range(NCORES)), trace=trace)
    outs = [
        np.asarray(res.results[i]["out"]).reshape(BPC, DIM, HH, WW)
        for i in range(NCORES)
    ]
    full = np.concatenate(outs, axis=0)
    return full, res


def kernel(**inputs) -> np.ndarray:
    full, _ = run(inputs, trace=False)
    return full


# revision 8
# speedup vs baseline: 1.3961x; 1.1272x over previous
"""Trainium2 Bass kernel for the two-branch sparse-attention fusion module.

Math (per batch b, tokens T = rgb/evt as (d=256, N=4096) d-major):
    s      = sum_n T[:, n]                           (256,)
    value[n] = T[:,n].v + c, v = (Wk^T Wq)^T s + N Wq^T bk, c = (Wk^T bq).s + N bq.bk
    w      = sigmoid((value_rgb - value_evt)/sqrt(d))
    out    = evt + w * (rgb - evt)

Engine split per batch (everything chases the DMA stream):
    DMA    : 4x (128,1024) block loads per tile, 2048-col block stores
    GpSimd : streaming row-sum partials (tensor_reduce X) during loads
    DVE    : M = A-D early (f32->bf16, overlapped with loads), M *= wb
             (bf16 x f32-PSUM), A = M+D (f32 out), tiny s-combines/v-adds
    PE     : tiny weight-product matvecs, fp32 value matmuls with bcast
             matmuls interleaved (K=1 bf16 broadcast of w to 128 partitions)
    ScalarE: sigmoid (bf16 out) only

Sharded data-parallel over batch: 8 cores x 2 batches, weights replicated.
"""

import numpy as np
from contextlib import ExitStack

import concourse.bass as bass
import concourse.tile as tile
from concourse import bacc, mybir
from concourse.bass_utils import run_bass_kernel_spmd

F32 = mybir.dt.float32
BF16 = mybir.dt.bfloat16

BS, DIM, HH, WW = 16, 256, 64, 64
N = HH * WW                 # 4096 tokens
NCORES = 8
BPC = BS // NCORES          # batches per core
PH = DIM // 128             # partition halves of the d dim
CH = 512                    # value-chunk (one PSUM bank of f32)
NCH = N // CH               # 8
LB = 1024                   # load block columns
NLB = N // LB               # 4
RB = 2048                   # reduce block columns
NRB = N // RB               # 2
SB = 2048                   # store/blend block columns
NSB = N // SB               # 2
INV_SQRT_D = 1.0 / 16.0


def build_nc() -> bass.Bass:
    nc = bacc.Bacc()

    rgb = nc.declare_dram_parameter("rgb", [BPC, PH, 128, N], F32, isOutput=False)
    evt = nc.declare_dram_parameter("evt", [BPC, PH, 128, N], F32, isOutput=False)
    wts = {}
    for nm in ("Wq_a", "Wk_a", "Wq_d", "Wk_d"):
        wts[nm] = nc.declare_dram_parameter(nm, [PH, 128, DIM], F32, isOutput=False)
    bss = {}
    for nm in ("bq_a", "bk_a", "bq_d", "bk_d"):
        bss[nm] = nc.declare_dram_parameter(nm, [PH, 128, 1], F32, isOutput=False)
    out = nc.declare_dram_parameter("out", [BPC, PH, 128, N], F32, isOutput=True)

    with tile.TileContext(nc) as tc:
        _body(tc, rgb, evt, wts, bss, out)
    nc.finalize()
    return nc


def _precompute(tc, ctx, consts, ps_sm, W, B):
    """Weight products; the d branch carries a folded minus sign."""
    nc = tc.nc
    PT, U, R = {}, {}, {}
    for br, wq, wk, sign in (
        ("a", "Wq_a", "Wk_a", 1.0),
        ("d", "Wq_d", "Wk_d", -1.0),
    ):
        for jh in range(PH):
            ps = ps_sm.tile([128, DIM], F32, tag="ps_sm", name=f"psPT{br}{jh}")
            for oh in range(PH):
                nc.tensor.matmul(
                    ps,
                    lhsT=W[(wk, oh)][:, jh * 128 : (jh + 1) * 128],
                    rhs=W[(wq, oh)],
                    start=(oh == 0),
                    stop=(oh == PH - 1),
                )
            t = consts.tile([128, DIM], F32, tag=f"PT{br}{jh}", name=f"PT{br}{jh}")
            nc.scalar.mul(out=t, in_=ps, mul=sign)
            PT[(br, jh)] = t
        for ih in range(PH):
            ps = ps_sm.tile([128, 1], F32, tag="ps_sm", name=f"psU{br}{ih}")
            for oh in range(PH):
                nc.tensor.matmul(
                    ps,
                    lhsT=W[(wq, oh)][:, ih * 128 : (ih + 1) * 128],
                    rhs=B[("bk_" + br, oh)],
                    start=(oh == 0),
                    stop=(oh == PH - 1),
                )
            t = consts.tile([128, 1], F32, tag=f"U{br}{ih}", name=f"U{br}{ih}")
            nc.scalar.mul(out=t, in_=ps, mul=sign * N)
            U[(br, ih)] = t
        for jh in range(PH):
            ps = ps_sm.tile([128, 1], F32, tag="ps_sm", name=f"psR{br}{jh}")
            for oh in range(PH):
                nc.tensor.matmul(
                    ps,
                    lhsT=W[(wk, oh)][:, jh * 128 : (jh + 1) * 128],
                    rhs=B[("bq_" + br, oh)],
                    start=(oh == 0),
                    stop=(oh == PH - 1),
                )
            t = consts.tile([128, 1], F32, tag=f"R{br}{jh}", name=f"R{br}{jh}")
            nc.scalar.mul(out=t, in_=ps, mul=sign)
            R[(br, jh)] = t

    # batch-independent bias-dot part of c_diff: N*(bq_a.bk_a - bq_d.bk_d)
    ps = ps_sm.tile([1, 1], F32, tag="ps_sm", name="psCb")
    k = 0
    for bq, bk, sgn in (("bq_a", "bk_a", 1), ("bq_d", "bk_d", -1)):
        for oh in range(PH):
            t = consts.tile([128, 1], F32, tag=f"bkN{bk}{oh}", name=f"bkN{bk}{oh}")
            nc.scalar.mul(out=t, in_=B[(bk, oh)], mul=float(sgn * N))
            nc.tensor.matmul(ps, lhsT=B[(bq, oh)], rhs=t, start=(k == 0), stop=(k == 3))
            k += 1
    c_bias = consts.tile([1, 1], F32, tag="c_bias")
    nc.scalar.copy(out=c_bias, in_=ps)
    return PT, U, R, c_bias


def _body(tc, rgb, evt, wts, bss, out):
    nc = tc.nc
    ACT = mybir.ActivationFunctionType
    with ExitStack() as ctx:
        consts = ctx.enter_context(tc.tile_pool(name="consts", bufs=1))
        data = ctx.enter_context(tc.tile_pool(name="data", bufs=2))
        mpool = ctx.enter_context(tc.tile_pool(name="mpool", bufs=2))
        small = ctx.enter_context(tc.tile_pool(name="small", bufs=2))
        wchunk = ctx.enter_context(tc.tile_pool(name="wchunk", bufs=4))
        ps_val = ctx.enter_context(tc.tile_pool(name="ps_val", bufs=3, space="PSUM"))
        ps_wb = ctx.enter_context(tc.tile_pool(name="ps_wb", bufs=3, space="PSUM"))
        ps_sm = ctx.enter_context(tc.tile_pool(name="ps_sm", bufs=2, space="PSUM"))

        # ---- load weights + biases -------------------------------------
        W = {}
        for nm in ("Wq_a", "Wk_a", "Wq_d", "Wk_d"):
            for h in range(PH):
                t = consts.tile([128, DIM], F32, tag=f"{nm}{h}", name=f"{nm}{h}")
                nc.sync.dma_start(out=t, in_=wts[nm][h])
                W[(nm, h)] = t
        B = {}
        for nm in ("bq_a", "bk_a", "bq_d", "bk_d"):
            for h in range(PH):
                t = consts.tile([128, 1], F32, tag=f"{nm}{h}", name=f"b{nm}{h}")
                nc.sync.dma_start(out=t, in_=bss[nm][h])
                B[(nm, h)] = t

        ones_row = consts.tile([1, 128], BF16, tag="ones")
        nc.vector.memset(ones_row, 1.0)
        one_one = consts.tile([1, 1], F32, tag="one_one")
        nc.vector.memset(one_one, 1.0)

        PT, U, R, c_bias = _precompute(tc, ctx, consts, ps_sm, W, B)

        garbage = consts.tile([128, 1], F32, tag="garbage")

        def emit_loads(b, st):
            A, Dv = {}, {}
            for h in range(PH):
                A[h] = data.tile([128, N], F32, tag=f"A{h}", name=f"A{h}_{b}")
                Dv[h] = data.tile([128, N], F32, tag=f"D{h}", name=f"D{h}_{b}")
                for blk in range(NLB):
                    sl = slice(blk * LB, (blk + 1) * LB)
                    nc.sync.dma_start(out=A[h][:, sl], in_=rgb[b, h][:, sl])
                    nc.sync.dma_start(out=Dv[h][:, sl], in_=evt[b, h][:, sl])
            st[b] = dict(A=A, Dv=Dv)

        def emit_subs(b, st):
            # M = A - D (bf16) on DVE
            A, Dv = st[b]["A"], st[b]["Dv"]
            M = {}
            for h in range(PH):
                M[h] = mpool.tile([128, N], BF16, tag=f"M{h}", name=f"M{h}_{b}")
                for rb in range(NRB):
                    sl = slice(rb * RB, (rb + 1) * RB)
                    nc.vector.tensor_sub(
                        out=M[h][:, sl], in0=A[h][:, sl], in1=Dv[h][:, sl]
                    )
            st[b]["M"] = M

        def make_red_ops(b, st):
            # row-sum partial ops on ScalarE (Copy + accum_out, discard main out)
            A, Dv = st[b]["A"], st[b]["Dv"]
            S4 = {}
            ops = []
            for key, tiles in (("a", A), ("d", Dv)):
                for h in range(PH):
                    s4 = small.tile(
                        [128, NRB], F32, tag=f"s4{key}{h}", name=f"s4{key}{h}_{b}"
                    )
                    S4[(key, h)] = s4
                    for rb in range(NRB):
                        sl = slice(rb * RB, (rb + 1) * RB)
                        ops.append(
                            lambda t=tiles[h], s=sl, dst=s4[:, rb : rb + 1]: (
                                nc.scalar.activation(
                                    out=garbage.broadcast_to([128, RB]),
                                    in_=t[:, s],
                                    func=ACT.Copy,
                                    accum_out=dst,
                                )
                            )
                        )
            st[b]["S4"] = S4
            return ops

        # ---- emission schedule ----------------------------------------
        st = [dict() for _ in range(BPC)]
        emit_loads(0, st)
        emit_subs(0, st)
        for op in make_red_ops(0, st):
            op()
        emit_loads(1, st)
        red1_ops = make_red_ops(1, st)

        def stage2(b, interleave_ops=(), emit_tail=None):
            A, Dv, M, S4 = st[b]["A"], st[b]["Dv"], st[b]["M"], st[b]["S4"]

            # combine row-sum partials (tiny, DVE)
            S = {}
            for key in ("a", "d"):
                for h in range(PH):
                    s = small.tile([128, 1], F32, tag=f"s{key}{h}", name=f"s{key}{h}_{b}")
                    nc.vector.reduce_sum(
                        out=s, in_=S4[(key, h)], axis=mybir.AxisListType.X
                    )
                    S[(key, h)] = s

            # v = PT @ s + U  per branch (d branch carries the minus sign)
            V = {}
            for br in ("a", "d"):
                for ih in range(PH):
                    ps = ps_sm.tile([128, 1], F32, tag="ps_sm", name=f"psv{br}{ih}_{b}")
                    for jh in range(PH):
                        nc.tensor.matmul(
                            ps,
                            lhsT=PT[(br, jh)][:, ih * 128 : (ih + 1) * 128],
                            rhs=S[(br, jh)],
                            start=(jh == 0),
                            stop=(jh == PH - 1),
                        )
                    v = small.tile([128, 1], F32, tag=f"v{br}{ih}", name=f"v{br}{ih}_{b}")
                    nc.vector.tensor_add(out=v, in0=ps, in1=U[(br, ih)])
                    V[(br, ih)] = v

            # c_diff = sum_j r[j] s[j] (both branches) + c_bias
            ps_c = ps_sm.tile([1, 1], F32, tag="ps_sm", name=f"psc_{b}")
            terms = [(S[(br, jh)], R[(br, jh)]) for br in ("a", "d") for jh in range(PH)]
            for i, (l, r) in enumerate(terms):
                nc.tensor.matmul(ps_c, lhsT=l, rhs=r, start=(i == 0), stop=False)
            nc.tensor.matmul(ps_c, lhsT=c_bias, rhs=one_one, start=False, stop=True)
            c16 = small.tile([1, 1], F32, tag="c16", name=f"c16_{b}")
            nc.scalar.mul(out=c16, in_=ps_c, mul=INV_SQRT_D)

            # value matmuls with interleaved w-broadcast matmuls on PE
            mms = [
                (V[("a", 0)], A[0]),
                (V[("a", 1)], A[1]),
                (V[("d", 0)], Dv[0]),
                (V[("d", 1)], Dv[1]),
            ]
            wb_ps = [None] * NCH

            def emit_value(ich):
                sl = slice(ich * CH, (ich + 1) * CH)
                psv = ps_val.tile([1, CH], F32, tag="psv", name=f"psval{ich}_{b}")
                for i, (v, t) in enumerate(mms):
                    nc.tensor.matmul(
                        psv, lhsT=v, rhs=t[:, sl],
                        start=(i == 0), stop=(i == len(mms) - 1),
                    )
                wrow = wchunk.tile([1, CH], BF16, tag="wrow", name=f"wrow{ich}_{b}")
                nc.scalar.activation(
                    out=wrow, in_=psv,
                    func=ACT.Sigmoid, bias=c16, scale=INV_SQRT_D,
                )
                return wrow

            def emit_bcast(ich, wrow):
                wb = ps_wb.tile([128, CH], F32, tag="wb", name=f"wb{ich}_{b}")
                nc.tensor.matmul(wb, lhsT=ones_row, rhs=wrow, start=True, stop=True)
                wb_ps[ich] = wb

            wrows = [None] * NCH
            for ich in range(NCH):
                wrows[ich] = emit_value(ich)
                if ich < len(interleave_ops):
                    interleave_ops[ich]()
                if ich >= 1:
                    emit_bcast(ich - 1, wrows[ich - 1])
            for op in interleave_ops[NCH:]:
                op()
            emit_bcast(NCH - 1, wrows[NCH - 1])

            # blend: M *= wb (bf16 x f32-psum), A = M + D (f32), store
            for h in range(PH):
                for ich in range(NCH):
                    sl = slice(ich * CH, (ich + 1) * CH)
                    nc.vector.tensor_mul(
                        out=M[h][:, sl], in0=M[h][:, sl], in1=wb_ps[ich]
                    )
            for h in range(PH):
                for sb in range(NSB):
                    sl = slice(sb * SB, (sb + 1) * SB)
                    nc.vector.tensor_add(
                        out=A[h][:, sl], in0=M[h][:, sl], in1=Dv[h][:, sl]
                    )
                    nc.sync.dma_start(out=out[b, h][:, sl], in_=A[h][:, sl])
            if emit_tail is not None:
                emit_tail()

        stage2(0, interleave_ops=red1_ops, emit_tail=lambda: emit_subs(1, st))
        stage2(1)


_NC_CACHE = None


def _get_nc():
    global _NC_CACHE
    if _NC_CACHE is None:
        _NC_CACHE = build_nc()
    return _NC_CACHE


def _make_in_maps(inputs):
    rgb = np.ascontiguousarray(np.asarray(inputs["rgb"], dtype=np.float32)).reshape(
        BS, PH, 128, N
    )
    evt = np.ascontiguousarray(np.asarray(inputs["evt"], dtype=np.float32)).reshape(
        BS, PH, 128, N
    )
    base = {}
    for nm in ("Wq_a", "Wk_a", "Wq_d", "Wk_d"):
        base[nm] = np.ascontiguousarray(
            np.asarray(inputs[nm], dtype=np.float32)
        ).reshape(PH, 128, DIM)
    for nm in ("bq_a", "bk_a", "bq_d", "bk_d"):
        base[nm] = np.ascontiguousarray(
            np.asarray(inputs[nm], dtype=np.float32)
        ).reshape(PH, 128, 1)
    in_maps = []
    for c in range(NCORES):
        m = dict(base)
        m["rgb"] = np.ascontiguousarray(rgb[c * BPC : (c + 1) * BPC])
        m["evt"] = np.ascontiguousarray(evt[c * BPC : (c + 1) * BPC])
        in_maps.append(m)
    return in_maps


def run(inputs, trace=False):
    nc = _get_nc()
    in_maps = _make_in_maps(inputs)
    res = run_bass_kernel_spmd(nc, in_maps, core_ids=list(range(NCORES)), trace=trace)
    outs = [
        np.asarray(res.results[i]["out"]).reshape(BPC, DIM, HH, WW)
        for i in range(NCORES)
    ]
    full = np.concatenate(outs, axis=0)
    return full, res


def kernel(**inputs) -> np.ndarray:
    full, _ = run(inputs, trace=False)
    return full
